# revision 12
# baseline (speedup 1.0000x reference)
"""Trainium2 Bass kernel for dynamic-scale FP8 GEMM (MixLinear):

    out = (scale_in * scale_w) * (q8(x / scale_in) @ q8(w).T) + bias
    scale_in = max|x| / 448  (global over the whole activation tensor)

Strategy (8 NeuronCores, SPMD, data-parallel over M = B*S = 16384):

  - The per-tensor activation scale is ONE scalar over an input the host
    already holds; it is computed host-side (exact fp16 |max| via a
    uint16 view) like the weight-side host prep (quant + packing), and
    shipped pre-broadcast as a [128, 2] f32 input.  No on-device amax,
    no AllGather: cores run fully independently, so the NEFF dispatch
    skew no longer rendezvous-stalls every core (the baseline lost ~70us
    to the barrier + collective + readback chain).
  - Weight is host-quantized to fp8 e4m3 (static scale 1.0 -> plain RNE
    cast; |w| << 240 so OCP e4m3fn bits == TRN fp8e4 bits), packed in
    k-PAIR order for the DoubleRow GEMM and grouped NT-MAJOR (4 groups
    of 4 n-tiles) so it streams in behind the GEMM's stationary-tile
    progression.
  - x pieces are loaded ROW-INTERLEAVED ("(p b) k -> p b k"): partition
    p takes b consecutive DRAM rows, so each descriptor is b*4KB
    contiguous (descriptor size sets per-queue DMA bandwidth).  The
    resulting m-column permutation is undone on the host during gather.
  - Quant (x * 224/amax -> fp8, values in +-224 < 240 TRN saturation,
    the 2x folds back into the dequant scale) runs on DVE
    (~1.3us/block).  The fp8 block is transposed on-chip by viewing
    adjacent fp8 k-PAIRS as one fp16 element (xbar transpose, half the
    bytes of an fp16 transpose).  CRITICAL measured fact: Tile
    serializes every xbar transpose against the other queue's in-flight
    DMAs (HW transpose-vs-DMA deadlock guard), so each transpose
    instruction is a global DMA barrier.  The transpose target is
    therefore BLOCK-MAJOR (xqT[p, mg, jj, 128]) so one instruction
    transposes a RANGE of adjacent blocks (contiguous 2D output): the
    whole kernel issues only 6 transpose instructions, pinned between
    load pieces with explicit ordering deps.
  - The GEMM runs m=128-column DoubleRow matmuls (the block-major rhs is
    contiguous per (mg, jj)): lead units cover single m-blocks 0-3 x
    nt-quarters in an order whose weight/x demand matches arrival;
    after that, units cover m-block PAIRS x nt-quarters with [P, 2, 128]
    psum tiles.  ~22 dummy fp8 matmuls during the load phase hold the
    PE's HAM clock gate open (8/8 = 2.4GHz; any idle window drops it to
    4/8, measured) so the real stream runs at full rate from t0.
  - PSUM eviction (out = psum*s2 + bias, fp16, output N-major) runs on
    ScalarE into [128, 4nt, m] tiles; ONE merged out-DMA per tile (40
    total, Sync queue, after the transposes -- no transpose‖copy
    exposure).  Per-core output is [N, M_shard]; the host un-permutes
    and transposes on gather.
"""

import os
import sys

try:
    import concourse  # noqa: F401
except ImportError:  # pragma: no cover
    for _p in ("/opt/trn_rl_repo", "/root/.axon_site/_ro/trn_rl_repo"):
        if os.path.isdir(_p) and _p not in sys.path:
            sys.path.insert(0, _p)

import ml_dtypes
import numpy as np

import concourse.bacc as bacc
import concourse.bass as bass  # noqa: F401
import concourse.mybir as mybir
import concourse.tile as tile
from concourse.bass_utils import run_bass_kernel_spmd

# Problem shapes (hardcoded per contract).
B, S, K, N = 4, 4096, 2048, 2048
M = B * S
N_CORES = 8
MS = M // N_CORES  # 2048 rows of x per core

P = 128
F16 = mybir.dt.float16
F32 = mybir.dt.float32
FP8 = mybir.dt.float8e4

NT_GROUPS = 4   # nt-major weight groups (4 n-tiles = 512 n columns each)
N_WARM = 22     # PE warm-up matmuls (HAM release before the first real mm)

# x load pieces (first block, n blocks): Sync carries blocks 0-9, Scalar
# carries w then blocks 10-15.  All pieces row-interleaved.
PIECES_SYNC = [(0, 2), (2, 2), (4, 2), (6, 2), (8, 2)]
PIECES_SCALAR = [(10, 6)]
# Transpose groups (first block, n blocks), one xbar instruction each.
TR_GROUPS = [(0, 2), (2, 2), (4, 2), (6, 2), (8, 2), (10, 6)]

# Lead GEMM units (single m-block, nt-quarter) ordered so weight-group /
# x-block demand matches arrival; then m-block-pair units sweep the rest.
LEAD_UNITS = [(0, 0), (1, 0), (0, 1), (1, 1), (0, 2), (1, 2), (2, 0), (3, 0),
              (0, 3), (1, 3), (2, 1), (3, 1), (2, 2), (3, 2), (2, 3), (3, 3)]
PAIRS = [(4, 5), (6, 7), (8, 9), (10, 11), (12, 13), (14, 15)]


def build_nc(ms=MS, k=K, n=N, n_cores=N_CORES):
    """Build + compile the per-core Bass program (SPMD: same NEFF on all cores)."""
    ko = k // P          # k planes (128 each)
    kj = ko // 2         # DoubleRow k steps (256 each)
    mg_n = ms // P       # m blocks (128 rows each)
    nt_tiles = n // P    # GEMM stationary n-tiles
    ntl = nt_tiles // NT_GROUPS  # n-tiles per weight group
    assert k % 256 == 0 and ms % 512 == 0 and n % 256 == 0

    nc = bacc.Bacc("TRN2", target_bir_lowering=False, debug=False, num_devices=n_cores)
    x = nc.dram_tensor("x", [ms, k], F16, kind="ExternalInput")
    wq8 = nc.dram_tensor("wq8", [P, NT_GROUPS * ko * (n // NT_GROUPS)], FP8,
                         kind="ExternalInput")
    b = nc.dram_tensor("b", [P, n // P], F16, kind="ExternalInput")
    sc = nc.dram_tensor("sc", [P, 2], F32, kind="ExternalInput")
    out_t = nc.dram_tensor("out_t", [n, ms], F16, kind="ExternalOutput")

    with tile.TileContext(nc) as tc:
        with (
            tc.tile_pool(name="big", bufs=1) as big,
            tc.tile_pool(name="small", bufs=1) as small,
            tc.tile_pool(name="ev", bufs=1) as ev,
            tc.tile_pool(name="psum", bufs=2, space="PSUM") as psum,
        ):
            # Persistent SBUF tensors.
            xnat = big.tile([P, mg_n, k], F16)   # x natural (row-interleaved blocks)
            xqn = big.tile([P, mg_n, k], FP8)    # quantized x, natural layout
            # BLOCK-MAJOR packed transpose target: fp16 element
            # [q, mg, jj, p] = fp8 pair (k = 2*(jj*128+q) + {0,1}) of
            # m-column mg*128+p.  A transpose of adjacent blocks writes a
            # CONTIGUOUS free range -> one instruction per block range.
            xqT = big.tile([P, mg_n, kj, P], F16)
            # w fp8, host packing: [p, g, h, nl] (nt-major groups)
            wq = big.tile([P, NT_GROUPS, ko, n // NT_GROUPS], FP8)

            # ---- Scales + bias (tiny, land first) -----------------------
            sc_bc = small.tile([P, 2], F32)
            nc.sync.dma_start(sc_bc[:], sc.ap())
            inv2s = sc_bc[:, 0:1]   # 224/amax  (quant scale)
            s2 = sc_bc[:, 1:2]      # amax/224  (dequant scale)

            bias16 = small.tile([P, nt_tiles], F16)
            nc.scalar.dma_start(bias16[:], b.ap())
            bias32 = small.tile([P, nt_tiles], F32)
            nc.vector.tensor_copy(bias32[:], bias16[:])

            # ---- PE warm-up: dummy fp8 DoubleRow matmuls ----------------
            warm_src = small.tile([P, 2, 512], FP8)
            nc.gpsimd.memset(warm_src[:], 0.0)
            # warm_ps shares the "ps" tag so its bank is recycled into the
            # GEMM's psum rotation once warm-up ends.
            warm_ps = psum.tile([P, 512], F32, tag="ps", bufs=8, name="warm_ps")
            for _ in range(N_WARM):
                nc.tensor.matmul(
                    warm_ps[:],
                    lhsT=warm_src[:, :, 0:P],
                    rhs=warm_src[:],
                    start=True,
                    stop=True,
                    perf_mode=mybir.MatmulPerfMode.DoubleRow,
                )

            # ---- Loads, quant (DVE), grouped packed transposes ----------
            xv = x.ap()
            wv = wq8.ap().rearrange("p (g r) -> p g r", g=NT_GROUPS)

            def load_x(eng, b0, nb):
                # Row-interleaved: partition p <- rows b0*128 + nb*p + j,
                # one nb*4KB contiguous descriptor per partition.
                return eng.dma_start(
                    out=xnat[:, b0:b0 + nb, :],
                    in_=xv[b0 * P:(b0 + nb) * P, :].rearrange(
                        "(p b) k2 -> p b k2", b=nb
                    ),
                )

            def quant(mg):
                nc.vector.tensor_scalar(
                    xqn[:, mg, :], xnat[:, mg, :], inv2s, None,
                    mybir.AluOpType.mult,
                )

            def transpose_group(b0, nb):
                # One xbar instruction for nb adjacent blocks: source
                # [128, nb*1024] f16 view, dest contiguous block-major.
                return nc.sync.dma_start(
                    out=xqT[:, b0:b0 + nb, :, :],
                    in_=xqn[:, b0:b0 + nb, :].bitcast(F16),
                    transpose=True,
                )

            # Scalar queue: weight groups, then x blocks 10-15.
            for g in range(NT_GROUPS):
                nc.scalar.dma_start(out=wq[:, g, :, :], in_=wv[:, g, :])
            for b0, nb in PIECES_SCALAR:
                load_x(nc.scalar, b0, nb)
            # Sync queue: x pieces with transpose groups PINNED between
            # them (the scheduler's DMA model is too optimistic and would
            # otherwise run every load ahead of the transposes).
            prev_tr = None
            for (b0, nb), (t0b, tnb) in zip(PIECES_SYNC, TR_GROUPS[:5]):
                li = load_x(nc.sync, b0, nb)
                if prev_tr is not None:
                    tile.add_dep_helper(
                        li.ins, prev_tr.ins, sync=False,
                        reason="pin transpose group before next x piece",
                    )
                for mg in range(b0, b0 + nb):
                    quant(mg)
                prev_tr = transpose_group(t0b, tnb)
            for mg in range(10, 16):
                quant(mg)

            # ---- GEMM (fp8 DoubleRow, m=128 columns) + fused eviction ---
            def rhs_ap(mg, jj):
                return (
                    xqT[:, mg, jj, :]
                    .bitcast(FP8)
                    .rearrange("p (m two) -> p two m", two=2)
                )

            def lhsT_ap(jj, nt):
                g, nl0 = divmod(nt, ntl)
                return wq[:, g, 2 * jj:2 * jj + 2, nl0 * P:(nl0 + 1) * P]

            def lead_unit(mg, q):
                # Single m-block, 4 stationary tiles, merged out-DMA.
                nt0 = q * 4
                ob = ev.tile([P, 4, P], F16, tag="obL", bufs=16,
                             name=f"obL_{mg}_{nt0}")
                for i in range(4):
                    nt = nt0 + i
                    ps = psum.tile([P, P], F32, tag="ps", bufs=8,
                                   name=f"psL_{mg}_{nt}")
                    for jj in range(kj):
                        nc.tensor.matmul(
                            ps[:], lhsT=lhsT_ap(jj, nt), rhs=rhs_ap(mg, jj),
                            start=(jj == 0), stop=(jj == kj - 1),
                            perf_mode=mybir.MatmulPerfMode.DoubleRow,
                        )
                    nc.scalar.activation(
                        ob[:, i, :], ps[:],
                        mybir.ActivationFunctionType.Identity,
                        bias=bias32[:, nt:nt + 1], scale=s2,
                    )
                nc.sync.dma_start(
                    out_t.ap()[nt0 * P:(nt0 + 4) * P, mg * P:(mg + 1) * P]
                    .rearrange("(i p) m -> p i m", i=4),
                    ob[:],
                )

            def pair_unit(mga, mgb, q):
                # m-block pair, 4 stationary tiles, merged out-DMA.
                nt0 = q * 4
                ob = ev.tile([P, 4, 2 * P], F16, tag="obP", bufs=5,
                             name=f"obP_{mga}_{nt0}")
                for i in range(4):
                    nt = nt0 + i
                    ps = psum.tile([P, 2, P], F32, tag="ps", bufs=8,
                                   name=f"psP_{mga}_{nt}")
                    for bi, mg in enumerate((mga, mgb)):
                        for jj in range(kj):
                            nc.tensor.matmul(
                                ps[:, bi, :], lhsT=lhsT_ap(jj, nt),
                                rhs=rhs_ap(mg, jj),
                                start=(jj == 0), stop=(jj == kj - 1),
                                perf_mode=mybir.MatmulPerfMode.DoubleRow,
                            )
                    nc.scalar.activation(
                        ob[:, i, :], ps[:],
                        mybir.ActivationFunctionType.Identity,
                        bias=bias32[:, nt:nt + 1], scale=s2,
                    )
                nc.sync.dma_start(
                    out_t.ap()[nt0 * P:(nt0 + 4) * P, mga * P:(mga + 2) * P]
                    .rearrange("(i p) m -> p i m", i=4),
                    ob[:],
                )

            for mg, q in LEAD_UNITS:
                lead_unit(mg, q)
            for pi, (mga, mgb) in enumerate(PAIRS):
                if pi == 2:
                    # blocks 10-15 transpose: pinned here so it takes its
                    # Sync slot after the early output drain, well before
                    # the (10,11) units need it.
                    transpose_group(*TR_GROUPS[5])
                for q in range(4):
                    pair_unit(mga, mgb, q)

    nc.compile()
    return nc


_NC_CACHE = {}


def _get_nc():
    if "nc" not in _NC_CACHE:
        _NC_CACHE["nc"] = build_nc()
    return _NC_CACHE["nc"]


def _col_of_row():
    """out_t column index for each x row (inverse of the row-interleaved
    load permutation): piece (b0, nb) puts x row b0*128 + nb*p + j into
    logical block b0+j at column position p."""
    col = np.empty(MS, dtype=np.int64)
    for b0, nb in PIECES_SYNC + PIECES_SCALAR:
        off = np.arange(nb * P)
        col[b0 * P + off] = (b0 + off % nb) * P + off // nb
    return col


def kernel(x, weight, bias):
    x = np.asarray(x, dtype=np.float16).reshape(M, K)
    weight = np.asarray(weight, dtype=np.float16)
    bias = np.asarray(bias, dtype=np.float16)

    nc = _get_nc()

    # Host-side dynamic per-tensor activation scale: exact amax of |x| via
    # the uint16 bit trick (for non-NaN fp16, ordering of (bits & 0x7fff)
    # matches ordering of |value|).  Mirrors the reference's f32
    # arithmetic: scale_ref = amax/448 (f32 RNE); the TRN fp8e4 grid is
    # driven with 2x that scale and the 2x folds back into the dequant
    # scale s2 = 2*scale_ref (exact).
    amax_bits = (x.view(np.uint16) & np.uint16(0x7FFF)).max()
    amax = np.float32(np.array(amax_bits, dtype=np.uint16).view(np.float16))
    scale_ref = np.maximum(amax / np.float32(448.0), np.float32(1e-12))
    s2 = scale_ref * np.float32(2.0)
    inv2s = np.float32(1.0) / s2
    sc = np.ascontiguousarray(
        np.broadcast_to(np.array([inv2s, s2], dtype=np.float32), (P, 2))
    )

    # Static-weight host prep: quantize (scale 1.0 -> plain RNE cast onto
    # the reference's e4m3fn grid; |w|<240 so bits == TRN fp8e4), transpose
    # to [K, N], and pack rows in k-PAIR order to match the on-chip packed
    # transpose: SBUF wq[q, pi, n] = w8T[k = (pi//2)*256 + 2q + (pi%2), n].
    # Additionally group n NT-MAJOR: [q, g, pi, nl] with n = g*512 + nl.
    w8T = weight.astype(np.float32).astype(ml_dtypes.float8_e4m3fn).T
    wq8 = (
        w8T.reshape(K // 256, 128, 2, N)        # [jj, q, pr, n]
        .transpose(0, 2, 1, 3)                  # [jj, pr, q, n] (pi = 2jj+pr)
        .reshape(K // P, P, N)                  # [pi, q, n]
        .transpose(1, 0, 2)                     # [q, pi, n]
        .reshape(P, K // P, NT_GROUPS, N // NT_GROUPS)  # [q, pi, g, nl]
        .transpose(0, 2, 1, 3)                  # [q, g, pi, nl]
        .reshape(P, K * N // P)
    )
    wq8 = np.ascontiguousarray(wq8)
    bias_pj = np.ascontiguousarray(bias.reshape(N // P, P).T)  # [p, j]
    in_maps = [
        {"x": x[c * MS:(c + 1) * MS], "wq8": wq8, "b": bias_pj, "sc": sc}
        for c in range(N_CORES)
    ]
    trace = bool(int(os.environ.get("KERNEL_TRACE", "0")))
    res = run_bass_kernel_spmd(nc, in_maps, list(range(N_CORES)), trace=trace)
    _NC_CACHE["last_result"] = res

    col = _col_of_row()
    out = np.empty((M, N), dtype=np.float16)
    for c in range(N_CORES):
        out[c * MS:(c + 1) * MS, :] = res.results[c]["out_t"][:, col].T
    return out.reshape(B, S, N)


# revision 15
# speedup vs baseline: 1.1733x; 1.1733x over previous
"""Trainium2 Bass kernel for dynamic-scale FP8 GEMM (MixLinear):

    out = (scale_in * scale_w) * (q8(x / scale_in) @ q8(w).T) + bias
    scale_in = max|x| / 448  (global over the whole activation tensor)

Strategy (8 NeuronCores, SPMD, data-parallel over M = B*S = 16384):

  - The per-tensor activation scale is ONE scalar over an input the host
    already holds; it is computed host-side (exact fp16 |max| via a
    uint16 view) like the weight-side host prep (quant + packing), and
    shipped pre-broadcast as a [128, 2] f32 input.  No on-device amax,
    no AllGather: cores run fully independently, so the NEFF dispatch
    skew no longer rendezvous-stalls every core (the baseline lost ~70us
    to the barrier + collective + readback chain).
  - Weight is host-quantized to fp8 e4m3 (static scale 1.0 -> plain RNE
    cast; |w| << 240 so OCP e4m3fn bits == TRN fp8e4 bits), packed in
    k-PAIR order for the DoubleRow GEMM and grouped NT-MAJOR (4 groups
    of 4 n-tiles) so it streams in behind the GEMM's stationary-tile
    progression.
  - x pieces are loaded ROW-INTERLEAVED ("(p b) k -> p b k"): partition
    p takes b consecutive DRAM rows, so each descriptor is b*4KB
    contiguous (descriptor size sets per-queue DMA bandwidth).  The
    resulting m-column permutation is undone on the host during gather.
  - Quant (x * 224/amax -> fp8, values in +-224 < 240 TRN saturation,
    the 2x folds back into the dequant scale) runs on DVE
    (~1.3us/block).  The fp8 block is transposed on-chip by viewing
    adjacent fp8 k-PAIRS as one fp16 element (xbar transpose, half the
    bytes of an fp16 transpose).  CRITICAL measured fact: Tile
    serializes every xbar transpose against the other queue's in-flight
    DMAs (HW transpose-vs-DMA deadlock guard), so each transpose
    instruction is a global DMA barrier.  The transpose target is
    therefore BLOCK-MAJOR (xqT[p, mg, jj, 128]) so one instruction
    transposes a RANGE of adjacent blocks (contiguous 2D output): the
    whole kernel issues only 6 transpose instructions, pinned between
    load pieces with explicit ordering deps.
  - The GEMM runs m=128-column DoubleRow matmuls (the block-major rhs is
    contiguous per (mg, jj)): lead units cover single m-blocks 0-3 x
    nt-quarters in an order whose weight/x demand matches arrival;
    after that, units cover m-block PAIRS x nt-quarters with [P, 2, 128]
    psum tiles.  ~22 dummy fp8 matmuls during the load phase hold the
    PE's HAM clock gate open (8/8 = 2.4GHz; any idle window drops it to
    4/8, measured) so the real stream runs at full rate from t0.
  - PSUM eviction (out = psum*s2 + bias, fp16, output N-major) runs on
    ScalarE into [128, 4nt, m] tiles; ONE merged out-DMA per tile (40
    total, Sync queue, after the transposes -- no transpose‖copy
    exposure).  Per-core output is [N, M_shard]; the host un-permutes
    and transposes on gather.
"""

import os
import sys

try:
    import concourse  # noqa: F401
except ImportError:  # pragma: no cover
    for _p in ("/opt/trn_rl_repo", "/root/.axon_site/_ro/trn_rl_repo"):
        if os.path.isdir(_p) and _p not in sys.path:
            sys.path.insert(0, _p)

import ml_dtypes
import numpy as np

import concourse.bacc as bacc
import concourse.bass as bass  # noqa: F401
import concourse.mybir as mybir
import concourse.tile as tile
from concourse.bass_utils import run_bass_kernel_spmd

# Problem shapes (hardcoded per contract).
B, S, K, N = 4, 4096, 2048, 2048
M = B * S
N_CORES = 8
MS = M // N_CORES  # 2048 rows of x per core

P = 128
F16 = mybir.dt.float16
F32 = mybir.dt.float32
FP8 = mybir.dt.float8e4

NT_GROUPS = 4   # nt-major weight groups (4 n-tiles = 512 n columns each)
N_WARM = 40     # PE warm-up matmuls (HAM release before the first real mm)

# x load pieces (first block, n blocks): Sync carries blocks 0-9 and
# 10-11, Scalar carries w then blocks 12-15.  All pieces row-interleaved.
PIECES_SYNC = [(0, 2), (2, 2), (4, 2), (6, 2), (8, 2), (10, 2)]
PIECES_SCALAR = [(12, 2), (14, 2)]
# Transpose groups (first block, n blocks), one xbar instruction each.
TR_GROUPS = [(0, 2), (2, 2), (4, 2), (6, 2), (8, 2), (10, 2), (12, 2), (14, 2)]

# GEMM units: (m-block-pair index, nt-quarter), ordered so weight-group
# and transpose demand matches the phase-locked arrival cadence.
PAIRS = [(0, 1), (2, 3), (4, 5), (6, 7), (8, 9), (10, 11), (12, 13), (14, 15)]
UNITS = [(0, 0), (0, 1), (0, 2), (1, 0), (1, 1), (0, 3), (1, 2), (1, 3)] + [
    (pi, q) for pi in range(2, 8) for q in range(4)
]


def build_nc(ms=MS, k=K, n=N, n_cores=N_CORES):
    """Build + compile the per-core Bass program (SPMD: same NEFF on all cores)."""
    ko = k // P          # k planes (128 each)
    kj = ko // 2         # DoubleRow k steps (256 each)
    mg_n = ms // P       # m blocks (128 rows each)
    nt_tiles = n // P    # GEMM stationary n-tiles
    ntl = nt_tiles // NT_GROUPS  # n-tiles per weight group
    assert k % 256 == 0 and ms % 512 == 0 and n % 256 == 0

    nc = bacc.Bacc("TRN2", target_bir_lowering=False, debug=False, num_devices=n_cores)
    x = nc.dram_tensor("x", [ms, k], F16, kind="ExternalInput")
    wq8 = nc.dram_tensor("wq8", [P, NT_GROUPS * ko * (n // NT_GROUPS)], FP8,
                         kind="ExternalInput")
    b = nc.dram_tensor("b", [P, n // P], F16, kind="ExternalInput")
    sc = nc.dram_tensor("sc", [P, 2], F32, kind="ExternalInput")
    out_t = nc.dram_tensor("out_t", [n, ms], F16, kind="ExternalOutput")

    with tile.TileContext(nc) as tc:
        with (
            tc.tile_pool(name="big", bufs=1) as big,
            tc.tile_pool(name="small", bufs=1) as small,
            tc.tile_pool(name="ev", bufs=1) as ev,
            tc.tile_pool(name="psum", bufs=2, space="PSUM") as psum,
        ):
            # Persistent SBUF tensors.
            xnat = big.tile([P, mg_n, k], F16)   # x natural (row-interleaved blocks)
            xqn = big.tile([P, mg_n, k], FP8)    # quantized x, natural layout
            # BLOCK-MAJOR packed transpose target: fp16 element
            # [q, mg, jj, p] = fp8 pair (k = 2*(jj*128+q) + {0,1}) of
            # m-column mg*128+p.  A transpose of adjacent blocks writes a
            # CONTIGUOUS free range -> one instruction per block range.
            xqT = big.tile([P, mg_n, kj, P], F16)
            # w fp8, host packing: [p, g, h, nl] (nt-major groups)
            wq = big.tile([P, NT_GROUPS, ko, n // NT_GROUPS], FP8)

            # ---- Scales + bias (tiny, land first) -----------------------
            sc_bc = small.tile([P, 2], F32)
            nc.sync.dma_start(sc_bc[:], sc.ap())
            inv2s = sc_bc[:, 0:1]   # 224/amax  (quant scale)
            s2 = sc_bc[:, 1:2]      # amax/224  (dequant scale)

            bias16 = small.tile([P, nt_tiles], F16)
            nc.scalar.dma_start(bias16[:], b.ap())
            bias32 = small.tile([P, nt_tiles], F32)
            nc.vector.tensor_copy(bias32[:], bias16[:])

            # ---- PE warm-up: dummy fp8 DoubleRow matmuls ----------------
            warm_src = small.tile([P, 2, 512], FP8)
            nc.gpsimd.memset(warm_src[:], 0.0)
            # warm_ps shares the "ps" tag so its bank is recycled into the
            # GEMM's psum rotation once warm-up ends.
            warm_ps = psum.tile([P, 512], F32, tag="ps", bufs=8, name="warm_ps")
            for _ in range(N_WARM):
                nc.tensor.matmul(
                    warm_ps[:],
                    lhsT=warm_src[:, :, 0:P],
                    rhs=warm_src[:],
                    start=True,
                    stop=True,
                    perf_mode=mybir.MatmulPerfMode.DoubleRow,
                )

            # ---- Loads, quant (DVE), grouped packed transposes ----------
            xv = x.ap()
            wv = wq8.ap().rearrange("p (g r) -> p g r", g=NT_GROUPS)

            def load_x(eng, b0, nb):
                # Row-interleaved: partition p <- rows b0*128 + nb*p + j,
                # one nb*4KB contiguous descriptor per partition.
                return eng.dma_start(
                    out=xnat[:, b0:b0 + nb, :],
                    in_=xv[b0 * P:(b0 + nb) * P, :].rearrange(
                        "(p b) k2 -> p b k2", b=nb
                    ),
                )

            def quant(mg):
                nc.vector.tensor_scalar(
                    xqn[:, mg, :], xnat[:, mg, :], inv2s, None,
                    mybir.AluOpType.mult,
                )

            def transpose_group(b0, nb):
                # One xbar instruction for nb adjacent blocks: source
                # [128, nb*1024] f16 view, dest contiguous block-major.
                return nc.sync.dma_start(
                    out=xqT[:, b0:b0 + nb, :, :],
                    in_=xqn[:, b0:b0 + nb, :].bitcast(F16),
                    transpose=True,
                )

            # Phase-locked load/transpose cadence.  Every xbar transpose
            # is a global DMA barrier (Tile serializes it against the
            # other queue's in-flight DMAs), so each Scalar piece is
            # dep-pinned behind a transpose: at any transpose's turn the
            # other queue has nothing in flight, and the pipeline
            # alternates load-phase / transpose cleanly.
            w_insts = []
            for g in range(NT_GROUPS):
                w_insts.append(nc.scalar.dma_start(out=wq[:, g, :, :],
                                                   in_=wv[:, g, :]))
            sc_x = [load_x(nc.scalar, b0, nb) for b0, nb in PIECES_SCALAR]
            tr = {}
            prev = None
            for b0, nb in PIECES_SYNC:
                li = load_x(nc.sync, b0, nb)
                if prev is not None:
                    tile.add_dep_helper(
                        li.ins, prev.ins, sync=False,
                        reason="pin transpose before next Sync x piece",
                    )
                for mg in range(b0, b0 + nb):
                    quant(mg)
                prev = tr[b0] = transpose_group(b0, nb)
            for b0, nb in PIECES_SCALAR:
                for mg in range(b0, b0 + nb):
                    quant(mg)
                prev = tr[b0] = transpose_group(b0, nb)
            # Scalar phase locks: w2 after TR(0-1), w3 after TR(2-3),
            # x12-13 after TR(6-7), x14-15 after TR(8-9).
            for ins_, tr_b0, why in (
                (w_insts[2], 0, "w2 after TR0-1"),
                (w_insts[3], 2, "w3 after TR2-3"),
                (sc_x[0], 6, "x12-13 after TR6-7"),
                (sc_x[1], 8, "x14-15 after TR8-9"),
            ):
                tile.add_dep_helper(ins_.ins, tr[tr_b0].ins, sync=False,
                                    reason=f"phase-lock: {why}")

            # ---- GEMM (fp8 DoubleRow, m=256 pair columns) + eviction ----
            def rhs_pair(mga, jj):
                # 4D moving AP: [p, two(pair), b(block), m] -> 256 columns
                # in (block, m) order from the block-major transpose target.
                return (
                    xqT[:, mga:mga + 2, jj, :]
                    .bitcast(FP8)
                    .rearrange("p b (m two) -> p two b m", two=2)
                )

            def lhsT_ap(jj, nt):
                g, nl0 = divmod(nt, ntl)
                return wq[:, g, 2 * jj:2 * jj + 2, nl0 * P:(nl0 + 1) * P]

            def pair_unit(mga, q):
                # m-block pair x 4 stationary tiles, merged out-DMA.
                nt0 = q * 4
                ob = ev.tile([P, 4, 2 * P], F16, tag="obP", bufs=15,
                             name=f"obP_{mga}_{nt0}")
                for i in range(4):
                    nt = nt0 + i
                    ps = psum.tile([P, 2 * P], F32, tag="ps", bufs=8,
                                   name=f"psP_{mga}_{nt}")
                    for jj in range(kj):
                        nc.tensor.matmul(
                            ps[:], lhsT=lhsT_ap(jj, nt),
                            rhs=rhs_pair(mga, jj),
                            start=(jj == 0), stop=(jj == kj - 1),
                            perf_mode=mybir.MatmulPerfMode.DoubleRow,
                        )
                    nc.scalar.activation(
                        ob[:, i, :], ps[:],
                        mybir.ActivationFunctionType.Identity,
                        bias=bias32[:, nt:nt + 1], scale=s2,
                    )
                nc.sync.dma_start(
                    out_t.ap()[nt0 * P:(nt0 + 4) * P, mga * P:(mga + 2) * P]
                    .rearrange("(i p) m -> p i m", i=4),
                    ob[:],
                )

            for pi, q in UNITS:
                pair_unit(PAIRS[pi][0], q)

    nc.compile()
    return nc


_NC_CACHE = {}


def _get_nc():
    if "nc" not in _NC_CACHE:
        _NC_CACHE["nc"] = build_nc()
    return _NC_CACHE["nc"]


def _col_of_row():
    """out_t column index for each x row (inverse of the row-interleaved
    load permutation): piece (b0, nb) puts x row b0*128 + nb*p + j into
    logical block b0+j at column position p."""
    col = np.empty(MS, dtype=np.int64)
    for b0, nb in PIECES_SYNC + PIECES_SCALAR:
        off = np.arange(nb * P)
        col[b0 * P + off] = (b0 + off % nb) * P + off // nb
    return col


def kernel(x, weight, bias):
    x = np.asarray(x, dtype=np.float16).reshape(M, K)
    weight = np.asarray(weight, dtype=np.float16)
    bias = np.asarray(bias, dtype=np.float16)

    nc = _get_nc()

    # Host-side dynamic per-tensor activation scale: exact amax of |x| via
    # the uint16 bit trick (for non-NaN fp16, ordering of (bits & 0x7fff)
    # matches ordering of |value|).  Mirrors the reference's f32
    # arithmetic: scale_ref = amax/448 (f32 RNE); the TRN fp8e4 grid is
    # driven with 2x that scale and the 2x folds back into the dequant
    # scale s2 = 2*scale_ref (exact).
    amax_bits = (x.view(np.uint16) & np.uint16(0x7FFF)).max()
    amax = np.float32(np.array(amax_bits, dtype=np.uint16).view(np.float16))
    scale_ref = np.maximum(amax / np.float32(448.0), np.float32(1e-12))
    s2 = scale_ref * np.float32(2.0)
    inv2s = np.float32(1.0) / s2
    sc = np.ascontiguousarray(
        np.broadcast_to(np.array([inv2s, s2], dtype=np.float32), (P, 2))
    )

    # Static-weight host prep: quantize (scale 1.0 -> plain RNE cast onto
    # the reference's e4m3fn grid; |w|<240 so bits == TRN fp8e4), transpose
    # to [K, N], and pack rows in k-PAIR order to match the on-chip packed
    # transpose: SBUF wq[q, pi, n] = w8T[k = (pi//2)*256 + 2q + (pi%2), n].
    # Additionally group n NT-MAJOR: [q, g, pi, nl] with n = g*512 + nl.
    w8T = weight.astype(np.float32).astype(ml_dtypes.float8_e4m3fn).T
    wq8 = (
        w8T.reshape(K // 256, 128, 2, N)        # [jj, q, pr, n]
        .transpose(0, 2, 1, 3)                  # [jj, pr, q, n] (pi = 2jj+pr)
        .reshape(K // P, P, N)                  # [pi, q, n]
        .transpose(1, 0, 2)                     # [q, pi, n]
        .reshape(P, K // P, NT_GROUPS, N // NT_GROUPS)  # [q, pi, g, nl]
        .transpose(0, 2, 1, 3)                  # [q, g, pi, nl]
        .reshape(P, K * N // P)
    )
    wq8 = np.ascontiguousarray(wq8)
    bias_pj = np.ascontiguousarray(bias.reshape(N // P, P).T)  # [p, j]
    in_maps = [
        {"x": x[c * MS:(c + 1) * MS], "wq8": wq8, "b": bias_pj, "sc": sc}
        for c in range(N_CORES)
    ]
    trace = bool(int(os.environ.get("KERNEL_TRACE", "0")))
    res = run_bass_kernel_spmd(nc, in_maps, list(range(N_CORES)), trace=trace)
    _NC_CACHE["last_result"] = res

    col = _col_of_row()
    out = np.empty((M, N), dtype=np.float16)
    for c in range(N_CORES):
        out[c * MS:(c + 1) * MS, :] = res.results[c]["out_t"][:, col].T
    return out.reshape(B, S, N)
